# revision 4
# baseline (speedup 1.0000x reference)
# Neural-collapse regularizer (tr_SW / tr_SB) on 8 TRN2 NeuronCores.
#
# Math: with per-class sums S_c = sum_{i: l_i=c} x_i, counts n_c,
# ssq = sum_i ||x_i||^2:
#   tr_SW = ssq - sum_c ||S_c||^2 / n_c
#   tr_SB = sum_c ||S_c/n_c - g||^2,  g = (sum_c S_c) / N
# So the device only needs the segment sums [C, D] and ssq; everything
# else is tiny O(C*D) host math.
#
# Sharding: class-parallel. Core k owns classes [128k, 128(k+1)); the host
# routes each row to the core that owns its label (segment sum is
# order-invariant so any within-core row order is fine). Each core then
# needs only a [128]-class one-hot per 128-row tile -> a single
# [128x128] x [128x512] matmul per tile, accumulated in PSUM.
# Per-row sum-of-squares s[p] is computed in one fused op (DVE
# tensor_tensor_reduce or ACT Square+accum_out, split across both
# engines) and folded into per-class ssq with an N=1 matmul that reuses
# the already-loaded one-hot weights.

import contextlib
import ctypes
import os
import sys
import types

import numpy as np
import ml_dtypes

import concourse.bass as bass
import concourse.bacc as bacc
import concourse.mybir as mybir
import concourse.tile as tile
from concourse.bass_utils import run_bass_kernel_spmd


def _ensure_ntff_hook():
    """Provide antenv.axon_hooks + an NTFF profile hook when the image's
    antenv package lacks it (needed only for trace=True timing runs)."""
    try:
        from antenv.axon_hooks import get_axon_ntff_profile_hook  # noqa: F401
        return
    except ImportError:
        pass
    mod = types.ModuleType("antenv.axon_hooks")
    state = {"hook": None}
    mod.set_axon_ntff_profile_hook = lambda h: state.__setitem__("hook", h)
    mod.get_axon_ntff_profile_hook = lambda: state["hook"]
    sys.modules["antenv.axon_hooks"] = mod

    so_path = "/opt/axon/libaxon_pjrt.so"
    if not os.path.exists(so_path):
        return
    lib = ctypes.CDLL(so_path)
    if not hasattr(lib, "axon_start_nrt_profile"):
        return
    lib.axon_start_nrt_profile.argtypes = [
        ctypes.POINTER(ctypes.c_int64), ctypes.c_size_t]
    lib.axon_start_nrt_profile.restype = ctypes.c_int64
    lib.axon_stop_nrt_profile.argtypes = [ctypes.c_char_p]
    lib.axon_stop_nrt_profile.restype = ctypes.c_int64

    @contextlib.contextmanager
    def _hook(output_dir, device_ids):
        import jax
        jax.devices()
        if device_ids:
            ids = (ctypes.c_int64 * len(device_ids))(*device_ids)
            rc = lib.axon_start_nrt_profile(ids, len(device_ids))
        else:
            rc = lib.axon_start_nrt_profile(None, 0)
        if rc != 0:
            raise RuntimeError(f"axon_start_nrt_profile rc={rc}")
        try:
            yield
        finally:
            n = lib.axon_stop_nrt_profile(str(output_dir).encode())
            print(f"profile: {n} file(s) written to {output_dir}",
                  file=sys.stderr)

    mod.set_axon_ntff_profile_hook(_hook)

CORES = 8
P = 128              # partitions = classes per core
D = 512              # feature dim (asserted against input)
GRP = 8              # 128-row tiles per DMA group
BF16 = mybir.dt.bfloat16
F32 = mybir.dt.float32
NP_BF16 = ml_dtypes.bfloat16

# which tiles within a group compute s on DVE (tensor_tensor_reduce);
# the rest use ACT (Square + accum_out). Tunable for engine balance.
DVE_JS = frozenset((0, 2, 4, 6))


def _host_shard(features: np.ndarray, labels: np.ndarray):
    """Route rows to cores by label, pad, bf16-cast, and lay out for DMA.

    Returns (in_maps, T) where in_maps[k] has:
      feat: [G, 128, GRP*D] bf16 -- group g, partition p, tile-in-group j
            holds row (g*GRP+j)*128 + p of the core's (padded) shard
      lab:  [128, T] f32 -- rebased label (0..127) of tile t, partition p
      iota: [128, 128] bf16 -- iota[p, f] = f
    """
    N, d = features.shape
    assert d == D, f"expected D={D}, got {d}"

    order = np.argsort(labels, kind="stable")
    sl = labels[order]
    # boundaries of each core's class range [128k, 128(k+1))
    bounds = np.searchsorted(sl, np.arange(0, CORES * P + 1, P))
    core_counts = np.diff(bounds)
    rows_per_core = int(np.ceil(core_counts.max() / (P * GRP)) * (P * GRP))
    T = rows_per_core // P
    G = T // GRP

    iota = np.broadcast_to(
        np.arange(P, dtype=NP_BF16), (P, P)
    ).copy()

    in_maps = []
    for k in range(CORES):
        idx = order[bounds[k]:bounds[k + 1]]
        n_k = len(idx)
        feat_k = np.zeros((rows_per_core, D), dtype=NP_BF16)
        feat_k[:n_k] = features[idx].astype(NP_BF16)
        lab_k = np.zeros((rows_per_core,), dtype=np.float32)
        lab_k[:n_k] = (labels[idx] - k * P).astype(np.float32)
        # [T*128, D] -> [G, GRP, 128, D] -> [G, 128, GRP, D] -> [G, 128, GRP*D]
        feat_k = np.ascontiguousarray(
            feat_k.reshape(G, GRP, P, D).transpose(0, 2, 1, 3).reshape(G, P, GRP * D)
        )
        lab_k = np.ascontiguousarray(lab_k.reshape(T, P).T)  # [128, T]
        in_maps.append({"feat": feat_k, "lab": lab_k, "iota": iota})
    return in_maps, T


def _build_program(T: int):
    G = T // GRP
    nc = bacc.Bacc("TRN2", target_bir_lowering=False, debug=False,
                   enable_asserts=False)
    feat_h = nc.dram_tensor("feat", [G, P, GRP * D], BF16, kind="ExternalInput")
    lab_h = nc.dram_tensor("lab", [P, T], F32, kind="ExternalInput")
    iota_h = nc.dram_tensor("iota", [P, P], BF16, kind="ExternalInput")
    out_h = nc.dram_tensor("out", [P, D + 1], F32, kind="ExternalOutput")

    with tile.TileContext(nc) as tc:
        with (
            tc.tile_pool(name="const", bufs=1) as constp,
            tc.tile_pool(name="x", bufs=3) as xp,
            tc.tile_pool(name="oh", bufs=3) as ohp,
            tc.tile_pool(name="sq", bufs=3) as sqp,
            tc.tile_pool(name="s", bufs=2) as sp,
            tc.tile_pool(name="outp", bufs=1) as outp,
            tc.tile_pool(name="psum", bufs=1, space="PSUM") as psump,
        ):
            iota_sb = constp.tile([P, P], BF16)
            nc.sync.dma_start(out=iota_sb[:], in_=iota_h[:])
            lab_sb = constp.tile([P, T], F32)
            nc.sync.dma_start(out=lab_sb[:], in_=lab_h[:])

            psum_sums = psump.tile([P, D], F32, tag="psum_sums")
            psum_ssqc = psump.tile([P, 1], F32, tag="psum_ssqc")

            for g in range(G):
                xg = xp.tile([P, GRP * D], BF16)
                nc.sync.dma_start(out=xg[:], in_=feat_h[g])
                ohg = ohp.tile([P, GRP * P], BF16)
                s8 = sp.tile([P, GRP], F32, tag="s8")
                for j in range(GRP):
                    t = g * GRP + j
                    xj = xg[:, j * D:(j + 1) * D]
                    oh = ohg[:, j * P:(j + 1) * P]
                    nc.vector.tensor_scalar(
                        oh, iota_sb[:], lab_sb[:, t:t + 1], None,
                        mybir.AluOpType.is_equal,
                    )
                    sq = sqp.tile([P, D], BF16, tag="sq")
                    if j in DVE_JS:
                        nc.vector.scalar_tensor_tensor(
                            out=sq[:], in0=xj, scalar=1.0, in1=xj,
                            op0=mybir.AluOpType.mult, op1=mybir.AluOpType.mult,
                            accum_out=s8[:, j:j + 1],
                        )
                    else:
                        nc.scalar.activation(
                            sq[:], xj, mybir.ActivationFunctionType.Square,
                            accum_out=s8[:, j:j + 1],
                        )
                    nc.tensor.matmul(
                        out=psum_sums[:], lhsT=oh, rhs=xj,
                        start=(t == 0), stop=(t == T - 1),
                    )
                sbf = sp.tile([P, GRP], BF16, tag="sbf")
                nc.vector.tensor_copy(out=sbf[:], in_=s8[:])
                for j in range(GRP):
                    t = g * GRP + j
                    oh = ohg[:, j * P:(j + 1) * P]
                    nc.tensor.matmul(
                        out=psum_ssqc[:], lhsT=oh, rhs=sbf[:, j:j + 1],
                        start=(t == 0), stop=(t == T - 1),
                    )

            out_sb = outp.tile([P, D + 1], F32)
            nc.vector.tensor_copy(out=out_sb[:, 0:D], in_=psum_sums[:])
            nc.vector.tensor_copy(out=out_sb[:, D:D + 1], in_=psum_ssqc[:])
            nc.sync.dma_start(out=out_h[:], in_=out_sb[:])

    nc.compile()
    return nc


def _finalize(results, labels: np.ndarray, C: int, N: int):
    sums = np.concatenate(
        [np.asarray(r["out"][:, :D], dtype=np.float64) for r in results], axis=0
    )  # [1024, D]
    ssq = float(sum(np.asarray(r["out"][:, D], dtype=np.float64).sum()
                    for r in results))
    counts = np.bincount(labels, minlength=CORES * P).astype(np.float64)

    sums = sums[:C]
    counts = counts[:C]
    means = sums / counts[:, None]
    g = sums.sum(axis=0) / N
    tr_sw = ssq - float((sums * sums).sum(axis=1).__truediv__(counts).sum())
    tr_sb = float(((means - g) ** 2).sum())
    return np.asarray(np.float32(tr_sw / tr_sb))


def run(features, labels, num_classes, trace=False):
    features = np.asarray(features, dtype=np.float32)
    labels = np.asarray(labels).astype(np.int64).ravel()
    C = int(num_classes)
    N = features.shape[0]
    assert C <= CORES * P, f"num_classes={C} exceeds {CORES * P}"

    if trace:
        _ensure_ntff_hook()
    in_maps, T = _host_shard(features, labels)
    nc = _build_program(T)
    res = run_bass_kernel_spmd(nc, in_maps, list(range(CORES)), trace=trace)
    out = _finalize(res.results, labels, C, N)
    return out, res


def kernel(**inputs) -> np.ndarray:
    trace = os.environ.get("KERNEL_TRACE", "0") == "1"
    out, _ = run(inputs["features"], inputs["labels"], inputs["num_classes"],
                 trace=trace)
    return out


# revision 6
# speedup vs baseline: 1.2107x; 1.2107x over previous
# Neural-collapse regularizer (tr_SW / tr_SB) on 8 TRN2 NeuronCores.
#
# Math: with per-class sums S_c = sum_{i: l_i=c} x_i, counts n_c,
# ssq = sum_i ||x_i||^2:
#   tr_SW = ssq - sum_c ||S_c||^2 / n_c
#   tr_SB = sum_c ||S_c/n_c - g||^2,  g = (sum_c S_c) / N
# So the device only needs the segment sums [C, D] and ssq; everything
# else is tiny O(C*D) host math.
#
# Sharding: class-parallel. Core k owns classes [128k, 128(k+1)); the host
# routes each row to the core that owns its label (segment sum is
# order-invariant so any within-core row order is fine).
#
# Layout trick: rows are packed in chunks of GRP=8 rows of a single class,
# one chunk per (group, partition) slot. All 8 row-tiles of a group then
# share one [128x128] one-hot (built once per group on DVE) and one
# stationary operand for all the group's matmuls.
#
# ssq trick: each row's sum-of-squares s[p] is produced by a fused
# square+row-reduce (DVE scalar_tensor_tensor or ACT Square+accum_out,
# split across both engines for balance) directly into a bf16 "s slot"
# at column 512 of the row's 514-wide lane. The second matmul of each
# tile covers columns [256, 513): its 257th output column accumulates
# per-class sum-of-squares in PSUM for free.

import contextlib
import ctypes
import os
import sys
import types

import numpy as np
import ml_dtypes

import concourse.bass as bass
import concourse.bacc as bacc
import concourse.mybir as mybir
import concourse.tile as tile
from concourse.bass_utils import run_bass_kernel_spmd


def _ensure_ntff_hook():
    """Provide antenv.axon_hooks + an NTFF profile hook when the image's
    antenv package lacks it (needed only for trace=True timing runs)."""
    try:
        from antenv.axon_hooks import get_axon_ntff_profile_hook  # noqa: F401
        return
    except ImportError:
        pass
    mod = types.ModuleType("antenv.axon_hooks")
    state = {"hook": None}
    mod.set_axon_ntff_profile_hook = lambda h: state.__setitem__("hook", h)
    mod.get_axon_ntff_profile_hook = lambda: state["hook"]
    sys.modules["antenv.axon_hooks"] = mod

    so_path = "/opt/axon/libaxon_pjrt.so"
    if not os.path.exists(so_path):
        return
    lib = ctypes.CDLL(so_path)
    if not hasattr(lib, "axon_start_nrt_profile"):
        return
    lib.axon_start_nrt_profile.argtypes = [
        ctypes.POINTER(ctypes.c_int64), ctypes.c_size_t]
    lib.axon_start_nrt_profile.restype = ctypes.c_int64
    lib.axon_stop_nrt_profile.argtypes = [ctypes.c_char_p]
    lib.axon_stop_nrt_profile.restype = ctypes.c_int64

    @contextlib.contextmanager
    def _hook(output_dir, device_ids):
        import jax
        jax.devices()
        if device_ids:
            ids = (ctypes.c_int64 * len(device_ids))(*device_ids)
            rc = lib.axon_start_nrt_profile(ids, len(device_ids))
        else:
            rc = lib.axon_start_nrt_profile(None, 0)
        if rc != 0:
            raise RuntimeError(f"axon_start_nrt_profile rc={rc}")
        try:
            yield
        finally:
            n = lib.axon_stop_nrt_profile(str(output_dir).encode())
            print(f"profile: {n} file(s) written to {output_dir}",
                  file=sys.stderr)

    mod.set_axon_ntff_profile_hook(_hook)


CORES = 8
P = 128              # partitions = classes per core
D = 512              # feature dim (asserted against input)
GRP = 8              # row-tiles per group = rows per chunk
LANE = D + 2         # per-tile lane: 512 features, 1 s-slot, 1 pad (align)
HALF = D // 2
BF16 = mybir.dt.bfloat16
F32 = mybir.dt.float32
NP_BF16 = ml_dtypes.bfloat16

# Fraction of tiles whose s is computed on DVE (scalar_tensor_tensor);
# the rest use ACT (Square + accum_out). Tunable for engine balance.
DVE_FRAC = float(os.environ.get("K_DVE_FRAC", "0.56"))


def _host_shard(features: np.ndarray, labels: np.ndarray):
    """Chunked class-sorted layout.

    Returns (in_maps, G). in_maps[k]:
      feat: [G, 128, GRP*LANE] bf16 -- slot (g, p) holds GRP rows of one
            class at j*LANE offsets; cols 512/513 of each lane are zero.
      lab:  [128, G] f32 -- rebased class (0..127) of slot (g, p)
      iota: [128, 128] bf16
    """
    N, d = features.shape
    assert d == D, f"expected D={D}, got {d}"
    CPAD = CORES * P

    order = np.argsort(labels, kind="stable")
    sl = labels[order]
    class_start = np.searchsorted(sl, np.arange(CPAD + 1))  # [1025]
    counts = np.diff(class_start)                            # [1024]
    chunks_per_class = -(-counts // GRP)                     # ceil
    core_chunks = chunks_per_class.reshape(CORES, P)
    G = int(-(-core_chunks.sum(axis=1).max() // P))

    fbf = features.astype(NP_BF16)
    iota = np.broadcast_to(np.arange(P, dtype=NP_BF16), (P, P)).copy()

    in_maps = []
    for k in range(CORES):
        nch = core_chunks[k]                    # chunks per rebased class
        total = int(nch.sum())
        assert total <= G * P
        # chunk m -> class: repeat
        chunk_cls = np.repeat(np.arange(P), nch)             # [total]
        # padded row grid: [G*P, GRP] of global row indices, -1 = empty
        grid = np.full((G * P, GRP), -1, dtype=np.int64)
        # scatter each class's rows into its chunks
        cls_pad_start = np.concatenate(([0], np.cumsum(nch * GRP)))  # [129]
        cnts = counts[k * P:(k + 1) * P]
        lo = class_start[k * P]
        n_k = int(cnts.sum())
        rows_k = order[lo:lo + n_k]
        lab_k = sl[lo:lo + n_k] - k * P          # rebased, sorted 0..127
        within = np.arange(n_k) - np.repeat(class_start[k * P:(k + 1) * P] - lo,
                                            cnts)
        pos = np.repeat(cls_pad_start[:-1], cnts) + within
        grid.reshape(-1)[pos] = rows_k

        # gather features; zero the padding rows
        safe = np.maximum(grid, 0)
        fr = fbf[safe.reshape(-1)]               # [G*P*GRP, D]
        fr[grid.reshape(-1) < 0] = 0
        fr = fr.reshape(G * P, GRP, D)

        feat = np.zeros((G * P, GRP, LANE), dtype=NP_BF16)
        feat[:, :, :D] = fr
        # chunk m -> (g = m // P, p = m % P)
        feat = feat.reshape(G, P, GRP * LANE)

        labg = np.zeros((G * P,), dtype=np.float32)
        labg[:total] = chunk_cls
        labg = np.ascontiguousarray(labg.reshape(G, P).T)    # [128, G]

        in_maps.append({"feat": feat, "lab": labg, "iota": iota})
    return in_maps, G


def _dve_flags(T: int):
    """Per-tile engine choice for s: True -> DVE, False -> ACT."""
    flags = []
    acc = 0.0
    for t in range(T):
        acc += DVE_FRAC
        if acc >= 1.0:
            acc -= 1.0
            flags.append(True)
        else:
            flags.append(False)
    return flags


def _build_program(G: int):
    T = G * GRP
    flags = _dve_flags(T)
    nc = bacc.Bacc("TRN2", target_bir_lowering=False, debug=False,
                   enable_asserts=False)
    feat_h = nc.dram_tensor("feat", [G, P, GRP * LANE], BF16,
                            kind="ExternalInput")
    lab_h = nc.dram_tensor("lab", [P, G], F32, kind="ExternalInput")
    iota_h = nc.dram_tensor("iota", [P, P], BF16, kind="ExternalInput")
    out_h = nc.dram_tensor("out", [P, D + 1], F32, kind="ExternalOutput")

    with tile.TileContext(nc) as tc:
        with (
            tc.tile_pool(name="const", bufs=1) as constp,
            tc.tile_pool(name="x", bufs=4) as xp,
            tc.tile_pool(name="oh", bufs=3) as ohp,
            tc.tile_pool(name="sqd", bufs=3) as sqdp,
            tc.tile_pool(name="sqa", bufs=3) as sqap,
            tc.tile_pool(name="outp", bufs=1) as outp,
            tc.tile_pool(name="psum", bufs=1, space="PSUM") as psump,
        ):
            iota_sb = constp.tile([P, P], BF16)
            nc.sync.dma_start(out=iota_sb[:], in_=iota_h[:])
            lab_sb = constp.tile([P, G], F32)
            nc.sync.dma_start(out=lab_sb[:], in_=lab_h[:])

            psum_a = psump.tile([P, HALF], F32, tag="psum_a")
            psum_b = psump.tile([P, HALF + 1], F32, tag="psum_b")

            with nc.allow_low_precision(
                    "bf16 per-row sum-of-squares; aggregate ssq error ~1e-5"):
                for g in range(G):
                    xg = xp.tile([P, GRP * LANE], BF16)
                    nc.sync.dma_start(out=xg[:], in_=feat_h[g])
                    oh = ohp.tile([P, P], BF16)
                    nc.vector.tensor_scalar(
                        oh[:], iota_sb[:], lab_sb[:, g:g + 1], None,
                        mybir.AluOpType.is_equal,
                    )
                    for j in range(GRP):
                        t = g * GRP + j
                        off = j * LANE
                        xj = xg[:, off:off + D]
                        s_slot = xg[:, off + D:off + D + 1]
                        if flags[t]:
                            sq = sqdp.tile([P, D], BF16, tag="sqd")
                            nc.vector.scalar_tensor_tensor(
                                out=sq[:], in0=xj, scalar=1.0, in1=xj,
                                op0=mybir.AluOpType.mult,
                                op1=mybir.AluOpType.mult,
                                accum_out=s_slot,
                            )
                        else:
                            sq = sqap.tile([P, D], BF16, tag="sqa")
                            nc.scalar.activation(
                                sq[:], xj,
                                mybir.ActivationFunctionType.Square,
                                accum_out=s_slot,
                            )
                    for j in range(GRP):
                        t = g * GRP + j
                        off = j * LANE
                        nc.tensor.matmul(
                            out=psum_a[:], lhsT=oh[:], rhs=xg[:, off:off + HALF],
                            start=(t == 0), stop=(t == T - 1),
                        )
                        nc.tensor.matmul(
                            out=psum_b[:], lhsT=oh[:],
                            rhs=xg[:, off + HALF:off + D + 1],
                            start=(t == 0), stop=(t == T - 1),
                        )

            out_sb = outp.tile([P, D + 1], F32)
            nc.vector.tensor_copy(out=out_sb[:, 0:HALF], in_=psum_a[:])
            nc.vector.tensor_copy(out=out_sb[:, HALF:D + 1], in_=psum_b[:])
            nc.sync.dma_start(out=out_h[:], in_=out_sb[:])

    nc.compile()
    return nc


def _finalize(results, labels: np.ndarray, C: int, N: int):
    sums = np.concatenate(
        [np.asarray(r["out"][:, :D], dtype=np.float64) for r in results], axis=0
    )  # [1024, D]
    ssq = float(sum(np.asarray(r["out"][:, D], dtype=np.float64).sum()
                    for r in results))
    counts = np.bincount(labels, minlength=CORES * P).astype(np.float64)

    sums = sums[:C]
    counts = counts[:C]
    means = sums / counts[:, None]
    g = sums.sum(axis=0) / N
    tr_sw = ssq - float(((sums * sums).sum(axis=1) / counts).sum())
    tr_sb = float(((means - g) ** 2).sum())
    return np.asarray(np.float32(tr_sw / tr_sb))


def run(features, labels, num_classes, trace=False):
    features = np.asarray(features, dtype=np.float32)
    labels = np.asarray(labels).astype(np.int64).ravel()
    C = int(num_classes)
    N = features.shape[0]
    assert C <= CORES * P, f"num_classes={C} exceeds {CORES * P}"

    if trace:
        _ensure_ntff_hook()
    in_maps, G = _host_shard(features, labels)
    nc = _build_program(G)
    res = run_bass_kernel_spmd(nc, in_maps, list(range(CORES)), trace=trace)
    out = _finalize(res.results, labels, C, N)
    return out, res


def kernel(**inputs) -> np.ndarray:
    trace = os.environ.get("KERNEL_TRACE", "0") == "1"
    out, _ = run(inputs["features"], inputs["labels"], inputs["num_classes"],
                 trace=trace)
    return out


# revision 9
# speedup vs baseline: 1.2241x; 1.0110x over previous
# Neural-collapse regularizer (tr_SW / tr_SB) on 8 TRN2 NeuronCores.
#
# Math: with per-class sums S_c = sum_{i: l_i=c} x_i, counts n_c,
# ssq = sum_i ||x_i||^2:
#   tr_SW = ssq - sum_c ||S_c||^2 / n_c
#   tr_SB = sum_c ||S_c/n_c - g||^2,  g = (sum_c S_c) / N
# So the device only needs the segment sums [C, D] and ssq; everything
# else is tiny O(C*D) host math.
#
# Sharding: class-parallel. Core k owns classes [128k, 128(k+1)); the host
# routes each row to the core that owns its label (segment sum is
# order-invariant so any within-core row order is fine).
#
# Layout trick: rows are packed in chunks of GRP=8 rows of a single class,
# one chunk per (group, partition) slot. All 8 row-tiles of a group then
# share one [128x128] one-hot (built once per group on DVE) and one
# stationary operand for all the group's matmuls.
#
# ssq trick: each row's sum-of-squares s[p] is produced by a fused
# square+row-reduce (DVE scalar_tensor_tensor or ACT Square+accum_out,
# split across both engines for balance) directly into a bf16 "s slot"
# at column 512 of the row's 514-wide lane. The second matmul of each
# tile covers columns [256, 513): its 257th output column accumulates
# per-class sum-of-squares in PSUM for free.

import contextlib
import ctypes
import os
import sys
import types

import numpy as np
import ml_dtypes

import concourse.bass as bass
import concourse.bacc as bacc
import concourse.mybir as mybir
import concourse.tile as tile
from concourse.bass_utils import run_bass_kernel_spmd


def _ensure_ntff_hook():
    """Provide antenv.axon_hooks + an NTFF profile hook when the image's
    antenv package lacks it (needed only for trace=True timing runs)."""
    try:
        from antenv.axon_hooks import get_axon_ntff_profile_hook  # noqa: F401
        return
    except ImportError:
        pass
    mod = types.ModuleType("antenv.axon_hooks")
    state = {"hook": None}
    mod.set_axon_ntff_profile_hook = lambda h: state.__setitem__("hook", h)
    mod.get_axon_ntff_profile_hook = lambda: state["hook"]
    sys.modules["antenv.axon_hooks"] = mod

    so_path = "/opt/axon/libaxon_pjrt.so"
    if not os.path.exists(so_path):
        return
    lib = ctypes.CDLL(so_path)
    if not hasattr(lib, "axon_start_nrt_profile"):
        return
    lib.axon_start_nrt_profile.argtypes = [
        ctypes.POINTER(ctypes.c_int64), ctypes.c_size_t]
    lib.axon_start_nrt_profile.restype = ctypes.c_int64
    lib.axon_stop_nrt_profile.argtypes = [ctypes.c_char_p]
    lib.axon_stop_nrt_profile.restype = ctypes.c_int64

    @contextlib.contextmanager
    def _hook(output_dir, device_ids):
        import jax
        jax.devices()
        if device_ids:
            ids = (ctypes.c_int64 * len(device_ids))(*device_ids)
            rc = lib.axon_start_nrt_profile(ids, len(device_ids))
        else:
            rc = lib.axon_start_nrt_profile(None, 0)
        if rc != 0:
            raise RuntimeError(f"axon_start_nrt_profile rc={rc}")
        try:
            yield
        finally:
            n = lib.axon_stop_nrt_profile(str(output_dir).encode())
            print(f"profile: {n} file(s) written to {output_dir}",
                  file=sys.stderr)

    mod.set_axon_ntff_profile_hook(_hook)


CORES = 8
P = 128              # partitions = classes per core
D = 512              # feature dim (asserted against input)
GRP = 8              # row-tiles per group = rows per chunk
LANE = D + 2         # per-tile lane: 512 features, 1 s-slot, 1 pad (align)
HALF = D // 2
BF16 = mybir.dt.bfloat16
F32 = mybir.dt.float32
NP_BF16 = ml_dtypes.bfloat16

# Fraction of tiles whose s is computed on DVE (scalar_tensor_tensor);
# the rest use ACT (Square + accum_out). Tunable for engine balance.
DVE_FRAC = float(os.environ.get("K_DVE_FRAC", "0.56"))


def _host_shard(features: np.ndarray, labels: np.ndarray):
    """Chunked class-sorted layout.

    Returns (in_maps, G). in_maps[k]:
      feat: [G, 128, GRP*LANE] bf16 -- slot (g, p) holds GRP rows of one
            class at j*LANE offsets; cols 512/513 of each lane are zero.
      lab:  [128, G] f32 -- rebased class (0..127) of slot (g, p)
      iota: [128, 128] bf16
    """
    N, d = features.shape
    assert d == D, f"expected D={D}, got {d}"
    CPAD = CORES * P

    order = np.argsort(labels, kind="stable")
    sl = labels[order]
    class_start = np.searchsorted(sl, np.arange(CPAD + 1))  # [1025]
    counts = np.diff(class_start)                            # [1024]
    chunks_per_class = -(-counts // GRP)                     # ceil
    core_chunks = chunks_per_class.reshape(CORES, P)
    G = int(-(-core_chunks.sum(axis=1).max() // P))

    fbf = features.astype(NP_BF16)
    iota = np.broadcast_to(np.arange(P, dtype=NP_BF16), (P, P)).copy()

    in_maps = []
    for k in range(CORES):
        nch = core_chunks[k]                    # chunks per rebased class
        total = int(nch.sum())
        assert total <= G * P
        # chunk m -> class: repeat
        chunk_cls = np.repeat(np.arange(P), nch)             # [total]
        # padded row grid: [G*P, GRP] of global row indices, -1 = empty
        grid = np.full((G * P, GRP), -1, dtype=np.int64)
        # scatter each class's rows into its chunks
        cls_pad_start = np.concatenate(([0], np.cumsum(nch * GRP)))  # [129]
        cnts = counts[k * P:(k + 1) * P]
        lo = class_start[k * P]
        n_k = int(cnts.sum())
        rows_k = order[lo:lo + n_k]
        lab_k = sl[lo:lo + n_k] - k * P          # rebased, sorted 0..127
        within = np.arange(n_k) - np.repeat(class_start[k * P:(k + 1) * P] - lo,
                                            cnts)
        pos = np.repeat(cls_pad_start[:-1], cnts) + within
        grid.reshape(-1)[pos] = rows_k

        # gather features; zero the padding rows
        safe = np.maximum(grid, 0)
        fr = fbf[safe.reshape(-1)]               # [G*P*GRP, D]
        fr[grid.reshape(-1) < 0] = 0
        fr = fr.reshape(G * P, GRP, D)

        feat = np.zeros((G * P, GRP, LANE), dtype=NP_BF16)
        feat[:, :, :D] = fr
        # chunk m -> (g = m // P, p = m % P)
        feat = feat.reshape(G, P, GRP * LANE)

        labg = np.zeros((G * P,), dtype=np.float32)
        labg[:total] = chunk_cls
        labg = np.ascontiguousarray(labg.reshape(G, P).T)    # [128, G]

        in_maps.append({"feat": feat, "lab": labg, "iota": iota})
    return in_maps, G


def _dve_flags(T: int):
    """Per-tile engine choice for s: True -> DVE, False -> ACT."""
    flags = []
    acc = 0.0
    for t in range(T):
        acc += DVE_FRAC
        if acc >= 1.0:
            acc -= 1.0
            flags.append(True)
        else:
            flags.append(False)
    return flags


def _build_program(G: int):
    T = G * GRP
    flags = _dve_flags(T)
    nc = bacc.Bacc("TRN2", target_bir_lowering=False, debug=False,
                   enable_asserts=False)
    feat_h = nc.dram_tensor("feat", [G, P, GRP * LANE], BF16,
                            kind="ExternalInput")
    lab_h = nc.dram_tensor("lab", [P, G], F32, kind="ExternalInput")
    iota_h = nc.dram_tensor("iota", [P, P], BF16, kind="ExternalInput")
    out_h = nc.dram_tensor("out", [P, D + 1], F32, kind="ExternalOutput")

    with tile.TileContext(nc) as tc:
        with (
            tc.tile_pool(name="const", bufs=1) as constp,
            tc.tile_pool(name="x", bufs=6) as xp,
            tc.tile_pool(name="oh", bufs=4) as ohp,
            tc.tile_pool(name="sqd", bufs=4) as sqdp,
            tc.tile_pool(name="sqa", bufs=4) as sqap,
            tc.tile_pool(name="outp", bufs=1) as outp,
            tc.tile_pool(name="psum", bufs=1, space="PSUM") as psump,
        ):
            # prefetch the first XBUFS feature groups before anything else so
            # the big DMAs start flowing immediately
            XBUFS = 6
            xgs = {}
            for g in range(min(XBUFS, G)):
                xgs[g] = xp.tile([P, GRP * LANE], BF16, name=f"xg{g}", tag="xg")
                nc.sync.dma_start(out=xgs[g][:], in_=feat_h[g])

            iota_sb = constp.tile([P, P], BF16)
            nc.sync.dma_start(out=iota_sb[:], in_=iota_h[:])
            lab_sb = constp.tile([P, G], F32)
            nc.sync.dma_start(out=lab_sb[:], in_=lab_h[:])

            psum_a = psump.tile([P, HALF], F32, tag="psum_a")
            psum_b = psump.tile([P, HALF + 1], F32, tag="psum_b")

            with nc.allow_low_precision(
                    "bf16 per-row sum-of-squares; aggregate ssq error ~1e-5"):
                for g in range(G):
                    if g in xgs:
                        xg = xgs[g]
                    else:
                        xg = xp.tile([P, GRP * LANE], BF16, name=f"xg{g}", tag="xg")
                        nc.sync.dma_start(out=xg[:], in_=feat_h[g])
                    oh = ohp.tile([P, P], BF16)
                    nc.vector.tensor_scalar(
                        oh[:], iota_sb[:], lab_sb[:, g:g + 1], None,
                        mybir.AluOpType.is_equal,
                    )
                    for j in range(GRP):
                        t = g * GRP + j
                        off = j * LANE
                        xj = xg[:, off:off + D]
                        s_slot = xg[:, off + D:off + D + 1]
                        if flags[t]:
                            sq = sqdp.tile([P, D], BF16, tag="sqd")
                            nc.vector.scalar_tensor_tensor(
                                out=sq[:], in0=xj, scalar=1.0, in1=xj,
                                op0=mybir.AluOpType.mult,
                                op1=mybir.AluOpType.mult,
                                accum_out=s_slot,
                            )
                        else:
                            sq = sqap.tile([P, D], BF16, tag="sqa")
                            nc.scalar.activation(
                                sq[:], xj,
                                mybir.ActivationFunctionType.Square,
                                accum_out=s_slot,
                            )
                    for j in range(GRP):
                        t = g * GRP + j
                        off = j * LANE
                        nc.tensor.matmul(
                            out=psum_a[:], lhsT=oh[:], rhs=xg[:, off:off + HALF],
                            start=(t == 0), stop=(t == T - 1),
                        )
                        nc.tensor.matmul(
                            out=psum_b[:], lhsT=oh[:],
                            rhs=xg[:, off + HALF:off + D + 1],
                            start=(t == 0), stop=(t == T - 1),
                        )

            out_sb = outp.tile([P, D + 1], F32)
            nc.vector.tensor_copy(out=out_sb[:, 0:HALF], in_=psum_a[:])
            nc.vector.tensor_copy(out=out_sb[:, HALF:D + 1], in_=psum_b[:])
            nc.sync.dma_start(out=out_h[:], in_=out_sb[:])

    nc.compile()
    return nc


def _finalize(results, labels: np.ndarray, C: int, N: int):
    sums = np.concatenate(
        [np.asarray(r["out"][:, :D], dtype=np.float64) for r in results], axis=0
    )  # [1024, D]
    ssq = float(sum(np.asarray(r["out"][:, D], dtype=np.float64).sum()
                    for r in results))
    counts = np.bincount(labels, minlength=CORES * P).astype(np.float64)

    sums = sums[:C]
    counts = counts[:C]
    means = sums / counts[:, None]
    g = sums.sum(axis=0) / N
    tr_sw = ssq - float(((sums * sums).sum(axis=1) / counts).sum())
    tr_sb = float(((means - g) ** 2).sum())
    return np.asarray(np.float32(tr_sw / tr_sb))


def run(features, labels, num_classes, trace=False):
    features = np.asarray(features, dtype=np.float32)
    labels = np.asarray(labels).astype(np.int64).ravel()
    C = int(num_classes)
    N = features.shape[0]
    assert C <= CORES * P, f"num_classes={C} exceeds {CORES * P}"

    if trace:
        _ensure_ntff_hook()
    in_maps, G = _host_shard(features, labels)
    nc = _build_program(G)
    res = run_bass_kernel_spmd(nc, in_maps, list(range(CORES)), trace=trace)
    out = _finalize(res.results, labels, C, N)
    return out, res


def kernel(**inputs) -> np.ndarray:
    trace = os.environ.get("KERNEL_TRACE", "0") == "1"
    out, _ = run(inputs["features"], inputs["labels"], inputs["num_classes"],
                 trace=trace)
    return out
